# revision 31
# baseline (speedup 1.0000x reference)
"""3-layer GCN (GCNConv x3, PyG defaults) on 8 Trainium2 NeuronCores.

Strategy (graph/data parallel, per sharding hint):
  - Nodes are sharded 8 ways by destination range (6250 nodes/core, padded to
    6272-token sections). Per layer, every core keeps ALL 50176 activation
    rows ("tokens": 128 x bf16 = 256 B) resident in SBUF.
  - A = D^-1/2 (A+I) D^-1/2 aggregation: per-edge gathers run on the GPSIMD
    dma_gather (SBUF source, transposed output [128 feat, edges]); the
    segment-sum runs on the TensorEngine as identity-matmul accumulation into
    PSUM, slot-major with nodes sorted by degree descending (slot k covers the
    prefix of nodes with degree > k) -- no padding waste.
  - dma_gather indices are int16, so sources are split into two passes
    (sections 0..3 / 4..7, re-based in_ap); the hi pass result (own node order
    sorted by hi-degree) is permuted into the lo pass's node order with a
    small on-chip token gather, then merged additively.
  - Dense stages (X@W, act@W) run on the TensorEngine with activations kept in
    [feat, node] layout == matmul lhsT layout. All deg^-1/2 scales fold into
    per-partition scalars of the PSUM-drain activation op.
  - Cross-core: one AllGather collective per layer redistributes the 6272
    freshly-computed p-rows per core (p = dinv * h).
All 8 cores run one identical program; only input data differs per core.
"""
import sys
import os

sys.path.insert(0, "/opt/trn_rl_repo")

import numpy as np
import ml_dtypes

from concourse import bass, bacc, mybir
from concourse import tile
from concourse.bass_utils import run_bass_kernel_spmd

BF16 = ml_dtypes.bfloat16
C = 8
BLK = 512
CHUNK = 4096
FOUT_PAD = 128  # W_out columns padded so layer-3 tokens share the 256B layout


# --------------------------------------------------------------------------
# Host-side preprocessing: pure integer/index work + normalization constants.
# --------------------------------------------------------------------------
class Plan:
    pass


def _pack_idx(vals):
    """int16 list -> [128, len/16] wrapped (i -> [i%16, i//16]) replicated x8."""
    n = len(vals)
    assert n % 16 == 0
    a = np.asarray(vals, dtype=np.int16).reshape(n // 16, 16).T  # [16, n/16]
    return np.tile(a, (8, 1))


def _pass_structure(jpos, toks, npc, dummy_tok):
    """Slot-major structure for one (core, pass).

    jpos: position (by this pass's degree-desc order) of each edge's dst.
    toks: gather token id of each edge's src.
    Returns (cells, data) where cells[(b, k)] = count and data[(b, k)] = token
    array (dst positions ascending within each cell).
    """
    # degree per position; positions with zero degree get one dummy edge
    cnt = np.bincount(jpos, minlength=npc)
    zpos = np.nonzero(cnt == 0)[0]
    if len(zpos):
        jpos = np.concatenate([jpos, zpos])
        toks = np.concatenate([toks, np.full(len(zpos), dummy_tok, np.int64)])
        cnt[zpos] = 1
    order = np.argsort(jpos, kind="stable")
    js = jpos[order]
    ts = toks[order]
    starts = np.zeros(npc, np.int64)
    starts[1:] = np.cumsum(cnt)[:-1]
    kk = np.arange(len(js)) - starts[js]
    bb = js // BLK
    o2 = np.lexsort((js, kk, bb))
    js, ts, kk, bb = js[o2], ts[o2], kk[o2], bb[o2]
    cells = {}
    data = {}
    # find cell boundaries
    cell_id = bb * 4096 + kk
    uniq, first = np.unique(cell_id, return_index=True)
    bounds = list(first) + [len(cell_id)]
    for i, u in enumerate(uniq):
        b, k = int(u) // 4096, int(u) % 4096
        seg = slice(bounds[i], bounds[i + 1])
        cells[(b, k)] = bounds[i + 1] - bounds[i]
        data[(b, k)] = ts[seg]
    return cells, data


def _chunk_cells(cells_max, nblk):
    """Pack (b,k) cells into gather chunks of <= CHUNK columns.

    Returns (chunks, cell_loc): chunks = list of padded lengths;
    cell_loc[(b,k)] = (chunk_idx, offset, n).
    """
    chunks = []
    cell_loc = {}
    cur = 0
    for b in range(nblk):
        ks = sorted(k for (bb, k) in cells_max if bb == b)
        for k in ks:
            n = cells_max[(b, k)]
            if cur == 0 or chunks[-1] + n > CHUNK:
                chunks.append(0)
                cur = 1
            cell_loc[(b, k)] = (len(chunks) - 1, chunks[-1], n)
            chunks[-1] += n
    # pad chunk lengths to multiples of 128
    chunks = [(l + 127) // 128 * 128 for l in chunks]
    return chunks, cell_loc


def preprocess(x, edge_index, n_nodes):
    p = Plan()
    N = n_nodes
    assert N % C == 0
    npc = N // C
    sec = (npc + 127) // 128 * 128
    p.npc, p.sec = npc, sec
    p.nranks = sec // 128
    p.nblk = (npc + BLK - 1) // BLK
    p.fin = x.shape[1]
    assert p.fin % 128 == 0
    p.finc = p.fin // 128

    src = np.asarray(edge_index[0], dtype=np.int64)
    dst = np.asarray(edge_index[1], dtype=np.int64)
    deg = (np.bincount(dst, minlength=N) + 1).astype(np.float32)
    dinv = (1.0 / np.sqrt(deg)).astype(np.float32)

    loops = np.arange(N, dtype=np.int64)
    src_all = np.concatenate([src, loops])
    dst_all = np.concatenate([dst, loops])
    s_core = src_all // npc
    d_core = dst_all // npc
    lo_cut = C // 2

    # per-core degree split and orders
    perm_lo = np.empty((C, npc), np.int64)
    perm_hi = np.empty((C, npc), np.int64)
    pos_lo = np.empty(N, np.int64)
    pos_hi_local = np.empty((C, npc), np.int64)
    deg_split = np.empty((C, 2, npc), np.int64)
    for m in range(C):
        sel = d_core == m
        dl = dst_all[sel] - m * npc
        lo = s_core[sel] < lo_cut
        cl = np.bincount(dl[lo], minlength=npc)
        ch = np.bincount(dl[~lo], minlength=npc)
        deg_split[m, 0], deg_split[m, 1] = cl, ch
        pl = np.argsort(-np.maximum(cl, 1), kind="stable")
        ph = np.argsort(-np.maximum(ch, 1), kind="stable")
        perm_lo[m], perm_hi[m] = pl, ph
        pos_lo[m * npc + pl] = np.arange(npc)
        pos_hi_local[m, ph] = np.arange(npc)
    p.perm_lo = perm_lo

    tok = (np.arange(N) // npc) * sec + pos_lo  # global token id per node
    half_tok = lo_cut * sec  # 25088: first hi token

    # structure per (core, pass): collect cells, then uniformize across cores
    all_cells = [[None] * C for _ in range(2)]
    all_data = [[None] * C for _ in range(2)]
    for m in range(C):
        sel = d_core == m
        sm_ = src_all[sel]
        dl = dst_all[sel] - m * npc
        lo = s_core[sel] < lo_cut
        for half in range(2):
            emask = lo if half == 0 else ~lo
            es, ed = sm_[emask], dl[emask]
            if half == 0:
                jp = pos_lo[m * npc + ed]
                tks = tok[es]
                dummy = npc  # token npc of section 0 (zero pad row)
            else:
                jp = pos_hi_local[m, ed]
                tks = tok[es] - half_tok
                dummy = npc
            cells, data = _pass_structure(jp, tks, npc, dummy)
            all_cells[half][m] = cells
            all_data[half][m] = data

    p.pass_chunks = []
    p.pass_cell_loc = []
    idx_arrays = [[None] * C for _ in range(2)]
    for half in range(2):
        cells_max = {}
        for m in range(C):
            for key, n in all_cells[half][m].items():
                cells_max[key] = max(cells_max.get(key, 0), n)
        chunks, cell_loc = _chunk_cells(cells_max, p.nblk)
        p.pass_chunks.append(chunks)
        p.pass_cell_loc.append(cell_loc)
        total = sum(chunks)
        for m in range(C):
            buf = np.full(total, npc, np.int16)  # default: dummy token
            coff = np.concatenate([[0], np.cumsum(chunks)])
            for key, (ci, off, n) in cell_loc.items():
                d = all_data[half][m].get(key)
                if d is not None:
                    buf[coff[ci] + off: coff[ci] + off + len(d)] = d.astype(np.int16)
            idx_arrays[half][m] = buf
    p.idx_lo = [_pack_idx(idx_arrays[0][m]) for m in range(C)]
    p.idx_hi = [_pack_idx(idx_arrays[1][m]) for m in range(C)]

    # perm gather: PB col i (pi_lo pos i) = TS token pos_hi(node at pi_lo pos i)
    p.idx_pm = []
    for m in range(C):
        pm = np.full(sec, npc, np.int64)
        pm[:npc] = pos_hi_local[m, perm_lo[m]]
        p.idx_pm.append(_pack_idx(pm.astype(np.int16)))

    # per-core dense inputs
    p.xT = []
    p.d1 = []
    p.d2 = []
    for m in range(C):
        pl = perm_lo[m]
        xm = np.asarray(x[m * npc:(m + 1) * npc][pl], dtype=np.float32)  # [npc, fin]
        xt = np.zeros((p.nranks, 128, p.finc, 128), BF16)
        xv = xm.reshape(npc, p.finc, 128).astype(BF16)
        flat = xt.reshape(sec, p.finc, 128)
        flat[:npc] = xv
        # layout [128 part, nranks, finc*128]: chunk c -> one DMA-able stripe
        p.xT.append(np.ascontiguousarray(xt.transpose(1, 0, 2, 3)))  # [128, nranks, finc, 128]
        dv = np.zeros(sec, np.float32)
        dv[:npc] = dinv[m * npc + pl]
        dvt = dv.reshape(p.nranks, 128).T.copy()  # [128, nranks]
        p.d1.append(dvt)
        p.d2.append(dvt * dvt)
    return p


# --------------------------------------------------------------------------
# Device kernel builder (one program, SPMD across 8 cores).
# --------------------------------------------------------------------------
def build_kernel(p, fmid, fout, bias_path=False, debug_stage=99, sim_mode=False):
    dt = mybir.dt
    nc = bacc.Bacc("TRN2", num_swdge_queues=1)
    sec, nranks, npc, nblk = p.sec, p.nranks, p.npc, p.nblk
    TOKR = C * nranks
    llo = sum(p.pass_chunks[0])
    lhi = sum(p.pass_chunks[1])

    xT_d = nc.dram_tensor("xT", [128, nranks, p.finc, 128], dt.bfloat16, kind="ExternalInput")
    w1_d = nc.dram_tensor("w1", [128, p.finc, fmid], dt.bfloat16, kind="ExternalInput")
    w2_d = nc.dram_tensor("w2", [fmid, fmid], dt.bfloat16, kind="ExternalInput")
    w3_d = nc.dram_tensor("w3", [fmid, FOUT_PAD], dt.bfloat16, kind="ExternalInput")
    d1_d = nc.dram_tensor("d1", [128, nranks], dt.float32, kind="ExternalInput")
    d2_d = nc.dram_tensor("d2", [128, nranks], dt.float32, kind="ExternalInput")
    id_d = nc.dram_tensor("ident", [128, 128], dt.bfloat16, kind="ExternalInput")
    il_d = nc.dram_tensor("idx_lo", [128, llo // 16], dt.int16, kind="ExternalInput")
    ih_d = nc.dram_tensor("idx_hi", [128, lhi // 16], dt.int16, kind="ExternalInput")
    ip_d = nc.dram_tensor("idx_pm", [128, sec // 16], dt.int16, kind="ExternalInput")
    out_d = nc.dram_tensor("out", [128, nranks * fout], dt.float32, kind="ExternalOutput")

    rh = nranks // 2
    ag_in0 = nc.dram_tensor("ag_in0", [128, rh * 128], dt.bfloat16)
    ag_in1 = nc.dram_tensor("ag_in1", [128, (nranks - rh) * 128], dt.bfloat16)
    ag_out0 = nc.dram_tensor("ag_out0", [C * 128, rh * 128], dt.bfloat16, addr_space="Shared")
    ag_out1 = nc.dram_tensor("ag_out1", [C * 128, (nranks - rh) * 128], dt.bfloat16, addr_space="Shared")

    with tile.TileContext(nc) as tc:
        with (
            tc.tile_pool(name="main", bufs=1) as main,
            tc.tile_pool(name="mp", bufs=6) as mp,
            tc.tile_pool(name="xc", bufs=3) as xcp,
            tc.tile_pool(name="sb", bufs=4) as sbp,
            tc.tile_pool(name="psb", bufs=3, space=bass.MemorySpace.PSUM) as psb,
            tc.tile_pool(name="pss", bufs=2, space=bass.MemorySpace.PSUM) as pss,
        ):
            P = main.tile([128, C, nranks, 128], dt.bfloat16)
            sA = main.tile([128, sec], dt.bfloat16)
            TS = main.tile([128, nranks, 128], dt.bfloat16)
            work = main.tile([128, 1, sec], dt.bfloat16)
            ident = main.tile([128, 128], dt.bfloat16)
            w1 = main.tile([128, p.finc, fmid], dt.bfloat16)
            w2 = main.tile([fmid, fmid], dt.bfloat16)
            w3 = main.tile([fmid, FOUT_PAD], dt.bfloat16)
            d1 = main.tile([128, nranks], dt.float32)
            d2 = main.tile([128, nranks], dt.float32)
            il = main.tile([128, llo // 16], dt.int16)
            ih = main.tile([128, lhi // 16], dt.int16)
            ip = main.tile([128, sec // 16], dt.int16)

            nc.sync.dma_start(ident[:], id_d[:])
            nc.sync.dma_start(w1[:], w1_d[:])
            nc.sync.dma_start(w2[:], w2_d[:])
            nc.sync.dma_start(w3[:], w3_d[:])
            nc.sync.dma_start(d1[:], d1_d[:])
            nc.sync.dma_start(d2[:], d2_d[:])
            nc.sync.dma_start(il[:], il_d[:])
            nc.sync.dma_start(ih[:], ih_d[:])
            nc.sync.dma_start(ip[:], ip_d[:])
            nc.gpsimd.memset(sA[:], 0.0)
            nc.gpsimd.memset(TS[:], 0.0)
            nc.gpsimd.memset(work[:], 0.0)

            def pstage_l1():
                for c in range(nranks):
                    xc = xcp.tile([128, p.finc, 128], dt.bfloat16, tag="xc")
                    nc.sync.dma_start(xc[:], xT_d[:, c, :, :])
                    ps = pss.tile([128, fmid], dt.float32, tag="pp")
                    for f in range(p.finc):
                        nc.tensor.matmul(ps[:], xc[:, f, :], w1[:, f, :], start=(f == 0),
                                         stop=(f == p.finc - 1))
                    nc.scalar.activation(work[:, 0, c * 128:(c + 1) * 128], ps[:],
                                         mybir.ActivationFunctionType.Copy,
                                         scale=d1[:, c:c + 1])

            def pstage(w, scale):
                for c in range(nranks):
                    ps = pss.tile([128, w.shape[-1]], dt.float32, tag="pp")
                    nc.tensor.matmul(ps[:], work[:, 0, c * 128:(c + 1) * 128], w[:],
                                     start=True, stop=True)
                    nc.scalar.activation(work[:, 0, c * 128:(c + 1) * 128],
                                         ps[:, 0:128] if w.shape[-1] >= 128 else ps[:],
                                         mybir.ActivationFunctionType.Copy,
                                         scale=scale[:, c:c + 1])

            def allgather():
                for hi, (agi, ago, a, b) in enumerate(
                        [(ag_in0, ag_out0, 0, rh), (ag_in1, ag_out1, rh, nranks)]):
                    w_ = (b - a) * 128
                    nc.sync.dma_start(agi[:, :], work[:, 0, a * 128:b * 128])
                    if not sim_mode:
                        nc.gpsimd.collective_compute(
                            "AllGather", mybir.AluOpType.bypass,
                            replica_groups=[list(range(C))],
                            ins=[agi.ap().opt()], outs=[ago.ap().opt()])
                    nc.sync.dma_start(
                        P[:, :, a:b, :],
                        ago.ap().rearrange("(c q) (r f) -> q c r f", q=128, f=128))

            def reduction(half, idxt, qbase, do_mm=True, do_drain=True):
                """One pass: gathers + identity-matmul psum accumulate.
                half 0 -> drains to sA; half 1 -> drains via transpose to TS."""
                chunks = p.pass_chunks[half]
                cell_loc = p.pass_cell_loc[half]
                in_ap = P[:, 0:C // 2, :, :] if half == 0 else P[:, C // 2:C, :, :]
                # slot counts per block
                kmax = {}
                for (b, k) in cell_loc:
                    kmax[b] = max(kmax.get(b, -1), k)
                coff = [0]
                for l in chunks:
                    coff.append(coff[-1] + l)
                mts = {}
                import os as _os
                _lim = int(_os.environ.get("GLIMIT", "9999"))
                for ci, clen in enumerate(chunks):
                    if ci >= _lim:
                        return
                    _ = None
                    m = mp.tile([128, 1, clen], dt.bfloat16, tag="m")
                    nc.gpsimd.dma_gather(
                        out_ap=m[:], in_ap=in_ap,
                        idxs_ap=idxt[:, coff[ci] // 16:(coff[ci] + clen) // 16],
                        num_idxs=clen, num_idxs_reg=clen, elem_size=128,
                        transpose=True, sbuf_tokens_per_rank=128,
                        sbuf_free_dim_per_rank=256, sbuf_free_dim_pad_per_rank=0,
                        sbuf_byte_offset=0, single_packet=False, queue_num=0)
                    mts[ci] = m
                if not do_mm:
                    return
                psd = {}
                for b in range(nblk):
                    bsz = min(BLK, npc - b * BLK)
                    ps = psb.tile([128, BLK], dt.float32, tag="ps")
                    psd[b] = ps
                    for k in range(kmax[b] + 1):
                        ci, off, n = cell_loc[(b, k)]
                        nc.tensor.matmul(ps[:, 0:n], ident[:], mts[ci][:, 0, off:off + n],
                                         start=(k == 0), stop=(k == kmax[b]))
                    if not do_drain:
                        continue
                    if half == 0:
                        nc.vector.tensor_copy(sA[:, b * BLK:b * BLK + bsz], ps[:, 0:bsz])
                    else:
                        sb = sbp.tile([128, BLK], dt.bfloat16, tag="sb")
                        nc.scalar.activation(sb[:, 0:bsz], ps[:, 0:bsz],
                                             mybir.ActivationFunctionType.Copy)
                        for q in range((bsz + 127) // 128):
                            w_ = min(128, bsz - q * 128)
                            pt = pss.tile([128, 128], dt.bfloat16, tag="pt")
                            nc.tensor.transpose(pt[0:w_, :], sb[:, q * 128:q * 128 + w_],
                                                ident[:])
                            r = (b * BLK) // 128 + q
                            nc.scalar.activation(TS[0:w_, r, :], pt[0:w_, :],
                                                 mybir.ActivationFunctionType.Copy)

            def perm_gather():
                nc.gpsimd.dma_gather(
                    out_ap=work[:], in_ap=TS[:], idxs_ap=ip[:],
                    num_idxs=sec, num_idxs_reg=sec, elem_size=128,
                    transpose=True, sbuf_tokens_per_rank=128,
                    sbuf_free_dim_per_rank=256, sbuf_free_dim_pad_per_rank=0,
                    sbuf_byte_offset=0, single_packet=False, queue_num=0)

            def merge_block(b, relu):
                a0 = b * BLK
                a1 = min(npc, (b + 1) * BLK)
                nc.vector.tensor_tensor(sA[:, a0:a1], sA[:, a0:a1],
                                        work[:, 0, a0:a1], mybir.AluOpType.add)
                if relu:
                    nc.scalar.activation(work[:, 0, a0:a1], sA[:, a0:a1],
                                         mybir.ActivationFunctionType.Relu)
                    if b == nblk - 1 and npc < sec:
                        nc.gpsimd.memset(work[:, 0, npc:sec], 0.0)

            def perm_merge(relu):
                perm_gather()
                for b in range(nblk):
                    merge_block(b, relu)

            def output_stage():
                sm = mp.tile([128, nranks, fout], dt.float32, tag="m")
                et = mp.tile([128, nranks, fout], dt.float32, tag="m")
                lg = xcp.tile([128, nranks], dt.float32, tag="xc")
                for c in range(nranks):
                    pt = pss.tile([128, 128], dt.bfloat16, tag="pt")
                    nc.tensor.transpose(pt[:], sA[:, c * 128:(c + 1) * 128], ident[:])
                    nc.scalar.activation(sm[:, c, :], pt[:, 0:fout],
                                         mybir.ActivationFunctionType.Copy,
                                         scale=d1[:, c:c + 1])
                nc.scalar.activation(et[:], sm[:], mybir.ActivationFunctionType.Exp)
                nc.vector.reduce_sum(lg[:], et[:], axis=mybir.AxisListType.X)
                nc.scalar.activation(lg[:], lg[:], mybir.ActivationFunctionType.Ln)
                for c in range(nranks):
                    nc.vector.tensor_scalar_sub(sm[:, c, :], sm[:, c, :], lg[:, c:c + 1])
                nc.sync.dma_start(out_d[:, :], sm[:].rearrange("q c f -> q (c f)"))

            # ---- program ----
            if debug_stage >= 1:
                pstage_l1()
            if debug_stage >= 1:
                allgather()
            if debug_stage >= 99:
                for layer in range(3):
                    reduction(1, ih, 2)
                    perm_gather()
                    reduction(0, il, 0)
                    for b in range(nblk):
                        merge_block(b, relu=(layer < 2))
                    if layer == 0:
                        pstage(w2, d2)
                        allgather()
                    elif layer == 1:
                        pstage(w3, d2)
                        allgather()
                    else:
                        output_stage()
            else:
                if debug_stage == 20:
                    reduction(0, il, 0, do_mm=False)
                elif debug_stage == 21:
                    reduction(0, il, 0, do_drain=False)
                elif debug_stage >= 2:
                    reduction(0, il, 0)
                if debug_stage >= 3:
                    reduction(1, ih, 2)
                if debug_stage >= 4:
                    perm_merge(relu=True)
                if debug_stage >= 5:
                    pstage(w2, d2)
                    allgather()
                sm = mp.tile([128, nranks, fout], dt.float32, tag="m")
                if debug_stage >= 4:
                    nc.scalar.activation(sm[:, :, 0:40], sA[:].rearrange("q (c f) -> q c f", c=nranks)[:, :, 0:40], mybir.ActivationFunctionType.Copy)
                else:
                    nc.gpsimd.memset(sm[:], 0.0)
                nc.sync.dma_start(out_d[:, :], sm[:].rearrange("q c f -> q (c f)"))
    nc.compile()
    return nc


# --------------------------------------------------------------------------
# Entry point
# --------------------------------------------------------------------------
def _make_in_maps(p, inputs, fmid, fout):
    W_in = np.asarray(inputs["W_in"], dtype=np.float32)
    W_mid = np.asarray(inputs["W_mid"], dtype=np.float32)
    W_out = np.asarray(inputs["W_out"], dtype=np.float32)
    w1 = np.ascontiguousarray(
        W_in.reshape(p.finc, 128, fmid).transpose(1, 0, 2).astype(BF16))
    w2 = np.ascontiguousarray(W_mid.astype(BF16))
    w3 = np.zeros((fmid, FOUT_PAD), BF16)
    w3[:, :fout] = W_out.astype(BF16)
    ident = np.eye(128, dtype=np.float32).astype(BF16)
    in_maps = []
    for m in range(C):
        in_maps.append({
            "xT": p.xT[m].reshape(128, p.nranks, p.finc, 128),
            "w1": w1, "w2": w2, "w3": w3,
            "d1": p.d1[m], "d2": p.d2[m], "ident": ident,
            "idx_lo": p.idx_lo[m], "idx_hi": p.idx_hi[m], "idx_pm": p.idx_pm[m],
        })
    return in_maps


def _run(inputs, trace=False, trace_cores=None, debug_stage=99):
    x = np.asarray(inputs["x"], dtype=np.float32)
    edge_index = np.asarray(inputs["edge_index"])
    W_in = np.asarray(inputs["W_in"], dtype=np.float32)
    W_mid = np.asarray(inputs["W_mid"], dtype=np.float32)
    W_out = np.asarray(inputs["W_out"], dtype=np.float32)
    for bname in ("b_in", "b_mid", "b_out"):
        if np.any(np.asarray(inputs[bname])):
            raise NotImplementedError("nonzero bias path not implemented")

    N, fin = x.shape
    fmid = W_in.shape[1]
    fout = W_out.shape[1]
    p = preprocess(x, edge_index, N)

    nc = build_kernel(p, fmid, fout, debug_stage=debug_stage)

    in_maps = _make_in_maps(p, inputs, fmid, fout)
    kw = {}
    if trace:
        kw = dict(trace=True, trace_cores=trace_cores or [0])
    r = run_bass_kernel_spmd(nc, in_maps, core_ids=list(range(C)), **kw)

    out = np.empty((N, fout), np.float32)
    for m in range(C):
        res = r.results[m]["out"]  # [128, nranks*fout] partition-major
        rows = res.reshape(128, p.nranks, fout).transpose(1, 0, 2).reshape(p.sec, fout)
        out[m * p.npc + p.perm_lo[m]] = rows[:p.npc]
    return out, r


def kernel(**inputs) -> np.ndarray:
    out, _ = _run(inputs)
    return out


# revision 32
# speedup vs baseline: 1.0002x; 1.0002x over previous
"""3-layer GCN (GCNConv x3, PyG defaults) on 8 Trainium2 NeuronCores.

Strategy (graph/data parallel, per sharding hint):
  - Nodes are sharded 8 ways by destination range (6250 nodes/core, padded to
    6272-token sections). Per layer, every core keeps ALL 50176 activation
    rows ("tokens": 128 x bf16 = 256 B) resident in SBUF.
  - A = D^-1/2 (A+I) D^-1/2 aggregation: per-edge gathers run on the GPSIMD
    dma_gather (SBUF source, transposed output [128 feat, edges]); the
    segment-sum runs on the TensorEngine as identity-matmul accumulation into
    PSUM, slot-major with nodes sorted by degree descending (slot k covers the
    prefix of nodes with degree > k) -- no padding waste.
  - dma_gather indices are int16, so sources are split into two passes
    (sections 0..3 / 4..7, re-based in_ap); the hi pass result (own node order
    sorted by hi-degree) is permuted into the lo pass's node order with a
    small on-chip token gather, then merged additively.
  - Dense stages (X@W, act@W) run on the TensorEngine with activations kept in
    [feat, node] layout == matmul lhsT layout. All deg^-1/2 scales fold into
    per-partition scalars of the PSUM-drain activation op.
  - Cross-core: one AllGather collective per layer redistributes the 6272
    freshly-computed p-rows per core (p = dinv * h).
All 8 cores run one identical program; only input data differs per core.
"""
import sys
import os

sys.path.insert(0, "/opt/trn_rl_repo")

import numpy as np
import ml_dtypes

from concourse import bass, bacc, mybir
from concourse import tile
from concourse.bass_utils import run_bass_kernel_spmd

BF16 = ml_dtypes.bfloat16
C = 8
BLK = 512
CHUNK = 4096
FOUT_PAD = 128  # W_out columns padded so layer-3 tokens share the 256B layout


# --------------------------------------------------------------------------
# Host-side preprocessing: pure integer/index work + normalization constants.
# --------------------------------------------------------------------------
class Plan:
    pass


def _pack_idx(vals):
    """int16 list -> [128, len/16] wrapped (i -> [i%16, i//16]) replicated x8."""
    n = len(vals)
    assert n % 16 == 0
    a = np.asarray(vals, dtype=np.int16).reshape(n // 16, 16).T  # [16, n/16]
    return np.tile(a, (8, 1))


def _pass_structure(jpos, toks, npc, dummy_tok):
    """Slot-major structure for one (core, pass).

    jpos: position (by this pass's degree-desc order) of each edge's dst.
    toks: gather token id of each edge's src.
    Returns (cells, data) where cells[(b, k)] = count and data[(b, k)] = token
    array (dst positions ascending within each cell).
    """
    # degree per position; positions with zero degree get one dummy edge
    cnt = np.bincount(jpos, minlength=npc)
    zpos = np.nonzero(cnt == 0)[0]
    if len(zpos):
        jpos = np.concatenate([jpos, zpos])
        toks = np.concatenate([toks, np.full(len(zpos), dummy_tok, np.int64)])
        cnt[zpos] = 1
    order = np.argsort(jpos, kind="stable")
    js = jpos[order]
    ts = toks[order]
    starts = np.zeros(npc, np.int64)
    starts[1:] = np.cumsum(cnt)[:-1]
    kk = np.arange(len(js)) - starts[js]
    bb = js // BLK
    o2 = np.lexsort((js, kk, bb))
    js, ts, kk, bb = js[o2], ts[o2], kk[o2], bb[o2]
    cells = {}
    data = {}
    # find cell boundaries
    cell_id = bb * 4096 + kk
    uniq, first = np.unique(cell_id, return_index=True)
    bounds = list(first) + [len(cell_id)]
    for i, u in enumerate(uniq):
        b, k = int(u) // 4096, int(u) % 4096
        seg = slice(bounds[i], bounds[i + 1])
        cells[(b, k)] = bounds[i + 1] - bounds[i]
        data[(b, k)] = ts[seg]
    return cells, data


def _chunk_cells(cells_max, nblk):
    """Pack (b,k) cells into gather chunks of <= CHUNK columns.

    Returns (chunks, cell_loc): chunks = list of padded lengths;
    cell_loc[(b,k)] = (chunk_idx, offset, n).
    """
    chunks = []
    cell_loc = {}
    cur = 0
    for b in range(nblk):
        ks = sorted(k for (bb, k) in cells_max if bb == b)
        for k in ks:
            n = cells_max[(b, k)]
            if cur == 0 or chunks[-1] + n > CHUNK:
                chunks.append(0)
                cur = 1
            cell_loc[(b, k)] = (len(chunks) - 1, chunks[-1], n)
            chunks[-1] += n
    # pad chunk lengths to multiples of 128
    chunks = [(l + 127) // 128 * 128 for l in chunks]
    return chunks, cell_loc


def preprocess(x, edge_index, n_nodes):
    p = Plan()
    N = n_nodes
    assert N % C == 0
    npc = N // C
    sec = (npc + 127) // 128 * 128
    p.npc, p.sec = npc, sec
    p.nranks = sec // 128
    p.nblk = (npc + BLK - 1) // BLK
    p.fin = x.shape[1]
    assert p.fin % 128 == 0
    p.finc = p.fin // 128

    src = np.asarray(edge_index[0], dtype=np.int64)
    dst = np.asarray(edge_index[1], dtype=np.int64)
    deg = (np.bincount(dst, minlength=N) + 1).astype(np.float32)
    dinv = (1.0 / np.sqrt(deg)).astype(np.float32)

    loops = np.arange(N, dtype=np.int64)
    src_all = np.concatenate([src, loops])
    dst_all = np.concatenate([dst, loops])
    s_core = src_all // npc
    d_core = dst_all // npc
    lo_cut = C // 2

    # per-core degree split and orders
    perm_lo = np.empty((C, npc), np.int64)
    perm_hi = np.empty((C, npc), np.int64)
    pos_lo = np.empty(N, np.int64)
    pos_hi_local = np.empty((C, npc), np.int64)
    deg_split = np.empty((C, 2, npc), np.int64)
    for m in range(C):
        sel = d_core == m
        dl = dst_all[sel] - m * npc
        lo = s_core[sel] < lo_cut
        cl = np.bincount(dl[lo], minlength=npc)
        ch = np.bincount(dl[~lo], minlength=npc)
        deg_split[m, 0], deg_split[m, 1] = cl, ch
        pl = np.argsort(-np.maximum(cl, 1), kind="stable")
        ph = np.argsort(-np.maximum(ch, 1), kind="stable")
        perm_lo[m], perm_hi[m] = pl, ph
        pos_lo[m * npc + pl] = np.arange(npc)
        pos_hi_local[m, ph] = np.arange(npc)
    p.perm_lo = perm_lo

    tok = (np.arange(N) // npc) * sec + pos_lo  # global token id per node
    half_tok = lo_cut * sec  # 25088: first hi token

    # structure per (core, pass): collect cells, then uniformize across cores
    all_cells = [[None] * C for _ in range(2)]
    all_data = [[None] * C for _ in range(2)]
    for m in range(C):
        sel = d_core == m
        sm_ = src_all[sel]
        dl = dst_all[sel] - m * npc
        lo = s_core[sel] < lo_cut
        for half in range(2):
            emask = lo if half == 0 else ~lo
            es, ed = sm_[emask], dl[emask]
            if half == 0:
                jp = pos_lo[m * npc + ed]
                tks = tok[es]
                dummy = npc  # token npc of section 0 (zero pad row)
            else:
                jp = pos_hi_local[m, ed]
                tks = tok[es] - half_tok
                dummy = npc
            cells, data = _pass_structure(jp, tks, npc, dummy)
            all_cells[half][m] = cells
            all_data[half][m] = data

    p.pass_chunks = []
    p.pass_cell_loc = []
    idx_arrays = [[None] * C for _ in range(2)]
    for half in range(2):
        cells_max = {}
        for m in range(C):
            for key, n in all_cells[half][m].items():
                cells_max[key] = max(cells_max.get(key, 0), n)
        chunks, cell_loc = _chunk_cells(cells_max, p.nblk)
        p.pass_chunks.append(chunks)
        p.pass_cell_loc.append(cell_loc)
        total = sum(chunks)
        for m in range(C):
            buf = np.full(total, npc, np.int16)  # default: dummy token
            coff = np.concatenate([[0], np.cumsum(chunks)])
            for key, (ci, off, n) in cell_loc.items():
                d = all_data[half][m].get(key)
                if d is not None:
                    buf[coff[ci] + off: coff[ci] + off + len(d)] = d.astype(np.int16)
            idx_arrays[half][m] = buf
    p.idx_lo = [_pack_idx(idx_arrays[0][m]) for m in range(C)]
    p.idx_hi = [_pack_idx(idx_arrays[1][m]) for m in range(C)]

    # perm gather: PB col i (pi_lo pos i) = TS token pos_hi(node at pi_lo pos i)
    p.idx_pm = []
    for m in range(C):
        pm = np.full(sec, npc, np.int64)
        pm[:npc] = pos_hi_local[m, perm_lo[m]]
        p.idx_pm.append(_pack_idx(pm.astype(np.int16)))

    # per-core dense inputs
    p.xT = []
    p.d1 = []
    p.d2 = []
    for m in range(C):
        pl = perm_lo[m]
        xm = np.asarray(x[m * npc:(m + 1) * npc][pl], dtype=np.float32)  # [npc, fin]
        xt = np.zeros((p.nranks, 128, p.finc, 128), BF16)
        xv = xm.reshape(npc, p.finc, 128).astype(BF16)
        flat = xt.reshape(sec, p.finc, 128)
        flat[:npc] = xv
        # layout [128 part, nranks, finc*128]: chunk c -> one DMA-able stripe
        p.xT.append(np.ascontiguousarray(xt.transpose(1, 0, 2, 3)))  # [128, nranks, finc, 128]
        dv = np.zeros(sec, np.float32)
        dv[:npc] = dinv[m * npc + pl]
        dvt = dv.reshape(p.nranks, 128).T.copy()  # [128, nranks]
        p.d1.append(dvt)
        p.d2.append(dvt * dvt)
    return p


# --------------------------------------------------------------------------
# Device kernel builder (one program, SPMD across 8 cores).
# --------------------------------------------------------------------------
def build_kernel(p, fmid, fout, bias_path=False, debug_stage=99, sim_mode=False):
    dt = mybir.dt
    nc = bacc.Bacc("TRN2", num_swdge_queues=1)
    sec, nranks, npc, nblk = p.sec, p.nranks, p.npc, p.nblk
    TOKR = C * nranks
    llo = sum(p.pass_chunks[0])
    lhi = sum(p.pass_chunks[1])

    xT_d = nc.dram_tensor("xT", [128, nranks, p.finc, 128], dt.bfloat16, kind="ExternalInput")
    w1_d = nc.dram_tensor("w1", [128, p.finc, fmid], dt.bfloat16, kind="ExternalInput")
    w2_d = nc.dram_tensor("w2", [fmid, fmid], dt.bfloat16, kind="ExternalInput")
    w3_d = nc.dram_tensor("w3", [fmid, FOUT_PAD], dt.bfloat16, kind="ExternalInput")
    d1_d = nc.dram_tensor("d1", [128, nranks], dt.float32, kind="ExternalInput")
    d2_d = nc.dram_tensor("d2", [128, nranks], dt.float32, kind="ExternalInput")
    id_d = nc.dram_tensor("ident", [128, 128], dt.bfloat16, kind="ExternalInput")
    il_d = nc.dram_tensor("idx_lo", [128, llo // 16], dt.int16, kind="ExternalInput")
    ih_d = nc.dram_tensor("idx_hi", [128, lhi // 16], dt.int16, kind="ExternalInput")
    ip_d = nc.dram_tensor("idx_pm", [128, sec // 16], dt.int16, kind="ExternalInput")
    out_d = nc.dram_tensor("out", [128, nranks * fout], dt.float32, kind="ExternalOutput")

    rh = nranks // 2
    ag_in0 = nc.dram_tensor("ag_in0", [128, rh * 128], dt.bfloat16)
    ag_in1 = nc.dram_tensor("ag_in1", [128, (nranks - rh) * 128], dt.bfloat16)
    ag_out0 = nc.dram_tensor("ag_out0", [C * 128, rh * 128], dt.bfloat16, addr_space="Shared")
    ag_out1 = nc.dram_tensor("ag_out1", [C * 128, (nranks - rh) * 128], dt.bfloat16, addr_space="Shared")

    with tile.TileContext(nc) as tc:
        with (
            tc.tile_pool(name="main", bufs=1) as main,
            tc.tile_pool(name="mp", bufs=6) as mp,
            tc.tile_pool(name="xc", bufs=3) as xcp,
            tc.tile_pool(name="sb", bufs=4) as sbp,
            tc.tile_pool(name="psb", bufs=3, space=bass.MemorySpace.PSUM) as psb,
            tc.tile_pool(name="pss", bufs=2, space=bass.MemorySpace.PSUM) as pss,
        ):
            P = main.tile([128, C, nranks, 128], dt.bfloat16)
            sA = main.tile([128, sec], dt.bfloat16)
            TS = main.tile([128, nranks, 128], dt.bfloat16)
            work = main.tile([128, 1, sec], dt.bfloat16)
            ident = main.tile([128, 128], dt.bfloat16)
            w1 = main.tile([128, p.finc, fmid], dt.bfloat16)
            w2 = main.tile([fmid, fmid], dt.bfloat16)
            w3 = main.tile([fmid, FOUT_PAD], dt.bfloat16)
            d1 = main.tile([128, nranks], dt.float32)
            d2 = main.tile([128, nranks], dt.float32)
            il = main.tile([128, llo // 16], dt.int16)
            ih = main.tile([128, lhi // 16], dt.int16)
            ip = main.tile([128, sec // 16], dt.int16)

            nc.sync.dma_start(ident[:], id_d[:])
            nc.sync.dma_start(w1[:], w1_d[:])
            nc.sync.dma_start(w2[:], w2_d[:])
            nc.sync.dma_start(w3[:], w3_d[:])
            nc.sync.dma_start(d1[:], d1_d[:])
            nc.sync.dma_start(d2[:], d2_d[:])
            nc.sync.dma_start(il[:], il_d[:])
            nc.sync.dma_start(ih[:], ih_d[:])
            nc.sync.dma_start(ip[:], ip_d[:])
            nc.vector.memset(sA[:], 0.0)
            nc.vector.memset(TS[:], 0.0)
            nc.vector.memset(work[:], 0.0)

            def pstage_l1():
                for c in range(nranks):
                    xc = xcp.tile([128, p.finc, 128], dt.bfloat16, tag="xc")
                    nc.sync.dma_start(xc[:], xT_d[:, c, :, :])
                    ps = pss.tile([128, fmid], dt.float32, tag="pp")
                    for f in range(p.finc):
                        nc.tensor.matmul(ps[:], xc[:, f, :], w1[:, f, :], start=(f == 0),
                                         stop=(f == p.finc - 1))
                    nc.scalar.activation(work[:, 0, c * 128:(c + 1) * 128], ps[:],
                                         mybir.ActivationFunctionType.Copy,
                                         scale=d1[:, c:c + 1])

            def pstage(w, scale):
                for c in range(nranks):
                    ps = pss.tile([128, w.shape[-1]], dt.float32, tag="pp")
                    nc.tensor.matmul(ps[:], work[:, 0, c * 128:(c + 1) * 128], w[:],
                                     start=True, stop=True)
                    nc.scalar.activation(work[:, 0, c * 128:(c + 1) * 128],
                                         ps[:, 0:128] if w.shape[-1] >= 128 else ps[:],
                                         mybir.ActivationFunctionType.Copy,
                                         scale=scale[:, c:c + 1])

            def allgather():
                for hi, (agi, ago, a, b) in enumerate(
                        [(ag_in0, ag_out0, 0, rh), (ag_in1, ag_out1, rh, nranks)]):
                    w_ = (b - a) * 128
                    nc.sync.dma_start(agi[:, :], work[:, 0, a * 128:b * 128])
                    if not sim_mode:
                        nc.gpsimd.collective_compute(
                            "AllGather", mybir.AluOpType.bypass,
                            replica_groups=[list(range(C))],
                            ins=[agi.ap().opt()], outs=[ago.ap().opt()])
                    nc.sync.dma_start(
                        P[:, :, a:b, :],
                        ago.ap().rearrange("(c q) (r f) -> q c r f", q=128, f=128))

            def reduction(half, idxt, qbase, do_mm=True, do_drain=True):
                """One pass: gathers + identity-matmul psum accumulate.
                half 0 -> drains to sA; half 1 -> drains via transpose to TS."""
                chunks = p.pass_chunks[half]
                cell_loc = p.pass_cell_loc[half]
                in_ap = P[:, 0:C // 2, :, :] if half == 0 else P[:, C // 2:C, :, :]
                # slot counts per block
                kmax = {}
                for (b, k) in cell_loc:
                    kmax[b] = max(kmax.get(b, -1), k)
                coff = [0]
                for l in chunks:
                    coff.append(coff[-1] + l)
                mts = {}
                import os as _os
                _lim = int(_os.environ.get("GLIMIT", "9999"))
                for ci, clen in enumerate(chunks):
                    if ci >= _lim:
                        return
                    _ = None
                    m = mp.tile([128, 1, clen], dt.bfloat16, tag="m")
                    nc.gpsimd.dma_gather(
                        out_ap=m[:], in_ap=in_ap,
                        idxs_ap=idxt[:, coff[ci] // 16:(coff[ci] + clen) // 16],
                        num_idxs=clen, num_idxs_reg=clen, elem_size=128,
                        transpose=True, sbuf_tokens_per_rank=128,
                        sbuf_free_dim_per_rank=256, sbuf_free_dim_pad_per_rank=0,
                        sbuf_byte_offset=0, single_packet=False, queue_num=0)
                    mts[ci] = m
                if not do_mm:
                    return
                psd = {}
                for b in range(nblk):
                    bsz = min(BLK, npc - b * BLK)
                    ps = psb.tile([128, BLK], dt.float32, tag="ps")
                    psd[b] = ps
                    for k in range(kmax[b] + 1):
                        ci, off, n = cell_loc[(b, k)]
                        nc.tensor.matmul(ps[:, 0:n], ident[:], mts[ci][:, 0, off:off + n],
                                         start=(k == 0), stop=(k == kmax[b]))
                    if not do_drain:
                        continue
                    if half == 0:
                        nc.vector.tensor_copy(sA[:, b * BLK:b * BLK + bsz], ps[:, 0:bsz])
                    else:
                        sb = sbp.tile([128, BLK], dt.bfloat16, tag="sb")
                        nc.scalar.activation(sb[:, 0:bsz], ps[:, 0:bsz],
                                             mybir.ActivationFunctionType.Copy)
                        for q in range((bsz + 127) // 128):
                            w_ = min(128, bsz - q * 128)
                            pt = pss.tile([128, 128], dt.bfloat16, tag="pt")
                            nc.tensor.transpose(pt[0:w_, :], sb[:, q * 128:q * 128 + w_],
                                                ident[:])
                            r = (b * BLK) // 128 + q
                            nc.scalar.activation(TS[0:w_, r, :], pt[0:w_, :],
                                                 mybir.ActivationFunctionType.Copy)

            def perm_gather():
                nc.gpsimd.dma_gather(
                    out_ap=work[:], in_ap=TS[:], idxs_ap=ip[:],
                    num_idxs=sec, num_idxs_reg=sec, elem_size=128,
                    transpose=True, sbuf_tokens_per_rank=128,
                    sbuf_free_dim_per_rank=256, sbuf_free_dim_pad_per_rank=0,
                    sbuf_byte_offset=0, single_packet=False, queue_num=0)

            def merge_block(b, relu):
                a0 = b * BLK
                a1 = min(npc, (b + 1) * BLK)
                nc.vector.tensor_tensor(sA[:, a0:a1], sA[:, a0:a1],
                                        work[:, 0, a0:a1], mybir.AluOpType.add)
                if relu:
                    nc.scalar.activation(work[:, 0, a0:a1], sA[:, a0:a1],
                                         mybir.ActivationFunctionType.Relu)
                    if b == nblk - 1 and npc < sec:
                        nc.vector.memset(work[:, 0, npc:sec], 0.0)

            def perm_merge(relu):
                perm_gather()
                for b in range(nblk):
                    merge_block(b, relu)

            def output_stage():
                sm = mp.tile([128, nranks, fout], dt.float32, tag="m")
                et = mp.tile([128, nranks, fout], dt.float32, tag="m")
                lg = xcp.tile([128, nranks], dt.float32, tag="xc")
                for c in range(nranks):
                    pt = pss.tile([128, 128], dt.bfloat16, tag="pt")
                    nc.tensor.transpose(pt[:], sA[:, c * 128:(c + 1) * 128], ident[:])
                    nc.scalar.activation(sm[:, c, :], pt[:, 0:fout],
                                         mybir.ActivationFunctionType.Copy,
                                         scale=d1[:, c:c + 1])
                nc.scalar.activation(et[:], sm[:], mybir.ActivationFunctionType.Exp)
                nc.vector.reduce_sum(lg[:], et[:], axis=mybir.AxisListType.X)
                nc.scalar.activation(lg[:], lg[:], mybir.ActivationFunctionType.Ln)
                for c in range(nranks):
                    nc.vector.tensor_scalar_sub(sm[:, c, :], sm[:, c, :], lg[:, c:c + 1])
                nc.sync.dma_start(out_d[:, :], sm[:].rearrange("q c f -> q (c f)"))

            # ---- program ----
            if debug_stage >= 1:
                pstage_l1()
            if debug_stage >= 1:
                allgather()
            if debug_stage >= 99:
                for layer in range(3):
                    reduction(1, ih, 2)
                    perm_gather()
                    reduction(0, il, 0)
                    for b in range(nblk):
                        merge_block(b, relu=(layer < 2))
                    if layer == 0:
                        pstage(w2, d2)
                        allgather()
                    elif layer == 1:
                        pstage(w3, d2)
                        allgather()
                    else:
                        output_stage()
            else:
                if debug_stage == 20:
                    reduction(0, il, 0, do_mm=False)
                elif debug_stage == 21:
                    reduction(0, il, 0, do_drain=False)
                elif debug_stage >= 2:
                    reduction(0, il, 0)
                if debug_stage >= 3:
                    reduction(1, ih, 2)
                if debug_stage >= 4:
                    perm_merge(relu=True)
                if debug_stage >= 5:
                    pstage(w2, d2)
                    allgather()
                sm = mp.tile([128, nranks, fout], dt.float32, tag="m")
                if debug_stage >= 4:
                    nc.scalar.activation(sm[:, :, 0:40], sA[:].rearrange("q (c f) -> q c f", c=nranks)[:, :, 0:40], mybir.ActivationFunctionType.Copy)
                else:
                    nc.gpsimd.memset(sm[:], 0.0)
                nc.sync.dma_start(out_d[:, :], sm[:].rearrange("q c f -> q (c f)"))
    nc.compile()
    return nc


# --------------------------------------------------------------------------
# Entry point
# --------------------------------------------------------------------------
def _make_in_maps(p, inputs, fmid, fout):
    W_in = np.asarray(inputs["W_in"], dtype=np.float32)
    W_mid = np.asarray(inputs["W_mid"], dtype=np.float32)
    W_out = np.asarray(inputs["W_out"], dtype=np.float32)
    w1 = np.ascontiguousarray(
        W_in.reshape(p.finc, 128, fmid).transpose(1, 0, 2).astype(BF16))
    w2 = np.ascontiguousarray(W_mid.astype(BF16))
    w3 = np.zeros((fmid, FOUT_PAD), BF16)
    w3[:, :fout] = W_out.astype(BF16)
    ident = np.eye(128, dtype=np.float32).astype(BF16)
    in_maps = []
    for m in range(C):
        in_maps.append({
            "xT": p.xT[m].reshape(128, p.nranks, p.finc, 128),
            "w1": w1, "w2": w2, "w3": w3,
            "d1": p.d1[m], "d2": p.d2[m], "ident": ident,
            "idx_lo": p.idx_lo[m], "idx_hi": p.idx_hi[m], "idx_pm": p.idx_pm[m],
        })
    return in_maps


def _run(inputs, trace=False, trace_cores=None, debug_stage=99):
    x = np.asarray(inputs["x"], dtype=np.float32)
    edge_index = np.asarray(inputs["edge_index"])
    W_in = np.asarray(inputs["W_in"], dtype=np.float32)
    W_mid = np.asarray(inputs["W_mid"], dtype=np.float32)
    W_out = np.asarray(inputs["W_out"], dtype=np.float32)
    for bname in ("b_in", "b_mid", "b_out"):
        if np.any(np.asarray(inputs[bname])):
            raise NotImplementedError("nonzero bias path not implemented")

    N, fin = x.shape
    fmid = W_in.shape[1]
    fout = W_out.shape[1]
    p = preprocess(x, edge_index, N)

    nc = build_kernel(p, fmid, fout, debug_stage=debug_stage)

    in_maps = _make_in_maps(p, inputs, fmid, fout)
    kw = {}
    if trace:
        kw = dict(trace=True, trace_cores=trace_cores or [0])
    r = run_bass_kernel_spmd(nc, in_maps, core_ids=list(range(C)), **kw)

    out = np.empty((N, fout), np.float32)
    for m in range(C):
        res = r.results[m]["out"]  # [128, nranks*fout] partition-major
        rows = res.reshape(128, p.nranks, fout).transpose(1, 0, 2).reshape(p.sec, fout)
        out[m * p.npc + p.perm_lo[m]] = rows[:p.npc]
    return out, r


def kernel(**inputs) -> np.ndarray:
    out, _ = _run(inputs)
    return out
